# revision 22
# baseline (speedup 1.0000x reference)
"""Trainium2 Bass kernel: per-channel exponential moving average.

  a_t = k*x_t + (1-k)*a_{t-1},  a_{-1} = x_0   (per batch, per channel)

Full inputs: x [16, 8000, 512] f32, smooth [512] f32. Output [16, 8000, 512].

Strategy (8 NeuronCores, data-parallel over batch, 2 batches/core), with a
phase-decomposed scan that spreads the recurrence over all four engines:

  - Host pre-scales u = k*x, transposes to channel-major and deinterleaves
    time by R=8 phases, fp16, laid out [C, NU, B_LOC, R, QU] so each
    (channel-group, q-chunk) unit is one DMA call with 16KB/partition
    contiguous descriptors.
  - PE: block sums z[c,q] = sum_m d_c^(R-1-m) * U_m[c,q] via 8 accumulating
    matmuls with diagonal stationaries diag(d^pow) into PSUM (per-partition
    scale-and-add at 1 col/cycle fp16).
  - DVE: tensor_tensor_scan only over the R-decimated series
    A[q] = d^R * A[q-1] + z[q]  (T/R elements per channel instead of T).
  - Recon chain per phase i: out_i = d * out_{i-1} + U_i, out_{-1} = Ashift.
    The scale runs on ACT (activation Copy, per-partition scale) except
    phase 0 on DVE; the add runs on DVE (tensor_tensor, 2x fp16) except
    phase 3 on GpSimd. Units are processed in software-interleaved groups
    of 3 so the cross-engine chain never head-blocks an in-order stream.
  - Output is stored per phase-half (8KB descriptors) so out tiles drain
    early; y layout is [C, NU, R, B_LOC, QU].
  - Host re-interleaves the fp16 output phases and upcasts to f32.
"""
import numpy as np
from contextlib import ExitStack

import concourse.bass as bass
from concourse import bacc, masks, mybir
import concourse.tile as tile
from concourse.bass_utils import run_bass_kernel_spmd

B, T, C = 16, 8000, 512
NCORES = 8
B_LOC = B // NCORES  # batches per core
P = 128
CG = C // P          # channel groups (4)
R = 8                # phase decimation factor
RH = R // 2          # phases per output half
Q = T // R           # decimated length (1000)
NU = 2               # q-chunks per channel group
QU = Q // NU         # 500 (fits one psum bank as f32)
F32 = mybir.dt.float32
F16 = mybir.dt.float16

_CACHED_NC = None


def _build_nc():
    nc = bacc.Bacc(None, target_bir_lowering=False)
    x = nc.declare_dram_parameter("x", [C, NU, B_LOC, R, QU], F16, isOutput=False)
    dpow_d = nc.declare_dram_parameter("dpow_d", [P, CG, R], F32, isOutput=False)
    dR_d = nc.declare_dram_parameter("dR_d", [P, CG], F32, isOutput=False)
    dcol = nc.declare_dram_parameter("dcol", [P, CG], F32, isOutput=False)
    x0t = nc.declare_dram_parameter("x0t", [P, CG, B_LOC, 1], F32, isOutput=False)
    y = nc.declare_dram_parameter("y", [C, NU, R, B_LOC, QU], F16, isOutput=True)

    with tile.TileContext(nc) as tc, ExitStack() as ctx:
        singles = ctx.enter_context(tc.tile_pool(name="singles", bufs=1))
        inpool = ctx.enter_context(tc.tile_pool(name="inpool", bufs=6))
        outpool = ctx.enter_context(tc.tile_pool(name="outpool", bufs=7))
        apool = ctx.enter_context(tc.tile_pool(name="apool", bufs=2))
        tmppool = ctx.enter_context(tc.tile_pool(name="tmppool", bufs=6))
        zpool = ctx.enter_context(tc.tile_pool(name="zpool", bufs=4, space="PSUM"))

        # small params ride the HWDGE (sync) queue; bulk xin leads SWDGE
        dpow_sb = singles.tile([P, CG, R], F32)
        nc.sync.dma_start(out=dpow_sb[:], in_=dpow_d[:])
        dR_sb = singles.tile([P, CG], F32)
        nc.sync.dma_start(out=dR_sb[:], in_=dR_d[:])
        dcol_sb = singles.tile([P, CG], F32)
        nc.sync.dma_start(out=dcol_sb[:], in_=dcol[:])
        x0_sb = singles.tile([P, CG, B_LOC, 1], F32)
        nc.sync.dma_start(out=x0_sb[:], in_=x0t[:])

        ident = singles.tile([P, P], F32)
        masks.make_identity(nc, ident[:])
        diag = singles.tile([P, CG, R, P], F16)
        for cg in range(CG):
            for m in range(R):
                nc.vector.tensor_scalar(
                    diag[:, cg, m, :], ident[:],
                    dpow_sb[:, cg, m : m + 1], None,
                    mybir.AluOpType.mult,
                )
        ones = singles.tile([P, QU], F32)
        nc.vector.memset(ones[:], 1.0)
        dRbc = singles.tile([P, CG, QU], F32)
        for cg in range(CG):
            nc.scalar.activation(
                dRbc[:, cg, :], ones[:],
                mybir.ActivationFunctionType.Copy,
                scale=dR_sb[:, cg : cg + 1],
            )

        units = [(cg, qc) for qc in range(NU) for cg in range(CG)]
        prev_A = {}

        def stage_front(cg, qc):
            """DMA in, PE z-accumulation, A carry slot, scans."""
            cs = slice(cg * P, (cg + 1) * P)
            xin = inpool.tile([P, B_LOC, R, QU], F16, tag="xin", name="xin")
            nc.gpsimd.dma_start(out=xin[:], in_=x[cs, qc, :, :, :])
            z = zpool.tile([P, B_LOC, 512], F32, tag="z", name="z")
            for m in range(R):
                for b in range(B_LOC):
                    nc.tensor.matmul(
                        z[:, b, 0:QU],
                        diag[:, cg, m, :],
                        xin[:, b, m, :],
                        start=(m == 0),
                        stop=(m == R - 1),
                    )
            A = apool.tile([P, B_LOC, 1 + QU], F16, tag=f"A{cg}", name=f"A{cg}")
            carry = (
                x0_sb[:, cg, :, :] if qc == 0 else prev_A[cg][:, :, QU : QU + 1]
            )
            nc.gpsimd.tensor_copy(A[:, :, 0:1], carry)
            for b in range(B_LOC):
                init = (
                    x0_sb[:, cg, b, :]
                    if qc == 0
                    else prev_A[cg][:, b, QU : QU + 1]
                )
                nc.vector.tensor_tensor_scan(
                    A[:, b, 1 : 1 + QU],
                    dRbc[:, cg, :],
                    z[:, b, 0:QU],
                    init,
                    mybir.AluOpType.mult,
                    mybir.AluOpType.add,
                )
            prev_A[cg] = A
            return [cs, xin, A, None, None]

        DVE_TS = {0}      # phases whose scale runs on DVE (4x tensor_scalar)
        GP_TT = {3}       # phases whose add runs on gpsimd

        def recon_phase(st, cg, qc, i):
            cs, xin, A, halves, last = st
            if halves is None:
                halves = [
                    outpool.tile([P, RH, B_LOC, QU], F16, tag="outh", name="outh")
                    for _ in range(2)
                ]
                st[3] = halves
            prev = A[:, :, 0:QU] if i == 0 else last
            half = halves[i // RH]
            cur = half[:, i % RH, :, :]
            tmp = tmppool.tile([P, B_LOC, QU], F16, tag="tmp", name="tmp")
            if i in DVE_TS:
                nc.vector.tensor_scalar(
                    tmp[:], prev, dcol_sb[:, cg : cg + 1], None,
                    mybir.AluOpType.mult,
                )
            else:
                nc.scalar.activation(
                    tmp[:], prev,
                    mybir.ActivationFunctionType.Copy,
                    scale=dcol_sb[:, cg : cg + 1],
                )
            eng = nc.gpsimd if i in GP_TT else nc.vector
            eng.tensor_tensor(cur, tmp[:], xin[:, :, i, :], mybir.AluOpType.add)
            st[4] = cur
            if i % RH == RH - 1:
                h = i // RH
                nc.gpsimd.dma_start(
                    out=y[cs, qc, h * RH : (h + 1) * RH, :, :], in_=half[:]
                )

        # software-interleaved groups of 3 units, pipelined one group ahead:
        # group g+1's fronts (DMA/PE/scans) are emitted before group g's
        # recon so the PE and DMA streams never pause for reconstruction
        groups = [units[0:3], units[3:6], units[6:8]]
        pending = None
        for grp in groups:
            sts = [(stage_front(cg, qc), cg, qc) for cg, qc in grp]
            if pending is not None:
                for i in range(R):
                    for st, cg, qc in pending:
                        recon_phase(st, cg, qc, i)
            pending = sts
        for i in range(R):
            for st, cg, qc in pending:
                recon_phase(st, cg, qc, i)
    nc.compile()
    return nc


def _get_nc():
    global _CACHED_NC
    if _CACHED_NC is None:
        _CACHED_NC = _build_nc()
    return _CACHED_NC


def _prep_in_maps(inputs, smooth):
    f16 = np.dtype("float16")
    x = np.asarray(inputs, dtype=np.float32)
    sm = np.asarray(smooth, dtype=np.float32)
    k = np.clip(sm, 0.0, 1.0).astype(np.float32)
    d = (1.0 - k).astype(np.float32)
    # U[c, qc, b, m, ql] = (k*x)[b, (qc*QU+ql)*R + m, c]
    kxT = (x * k[None, None, :]).transpose(0, 2, 1)  # [B, C, T]
    U = np.ascontiguousarray(
        kxT.reshape(B, C, NU, QU, R).transpose(1, 2, 0, 4, 3)
    ).astype(f16)  # [C, NU, B, R, QU]
    dcol = np.ascontiguousarray(d.reshape(CG, P).T)  # [P, CG]
    d64 = d.astype(np.float64)
    # dpow[p, cg, m] = d_c^(R-1-m)
    pw = np.stack([d64 ** (R - 1 - m) for m in range(R)], axis=1)  # [C, R]
    dpow = np.ascontiguousarray(
        pw.astype(np.float32).reshape(CG, P, R).transpose(1, 0, 2)
    )
    dR = np.ascontiguousarray((d64 ** R).astype(np.float32).reshape(CG, P).T)
    x0 = x[:, 0, :].T.reshape(CG, P, B).transpose(1, 0, 2)[..., None]
    return [
        {
            "x": np.ascontiguousarray(U[:, :, i * B_LOC : (i + 1) * B_LOC]),
            "dpow_d": dpow,
            "dR_d": dR,
            "dcol": dcol,
            "x0t": np.ascontiguousarray(x0[:, :, i * B_LOC : (i + 1) * B_LOC, :]),
        }
        for i in range(NCORES)
    ]


def _install_ntff_shim():
    """Provide antenv.axon_hooks if the image lacks it (trace=True path).

    Replicates trn_agent_boot's ctypes NTFF hook against libaxon_pjrt.so.
    """
    import sys

    if "antenv.axon_hooks" in sys.modules:
        return
    try:
        import antenv.axon_hooks  # noqa: F401
        return
    except ImportError:
        pass
    import contextlib
    import ctypes
    import types

    so_path = "/opt/axon/libaxon_pjrt.so"
    try:
        lib = ctypes.CDLL(so_path)
    except OSError:
        return
    if not hasattr(lib, "axon_start_nrt_profile"):
        return
    lib.axon_start_nrt_profile.argtypes = [
        ctypes.POINTER(ctypes.c_int64),
        ctypes.c_size_t,
    ]
    lib.axon_start_nrt_profile.restype = ctypes.c_int64
    lib.axon_stop_nrt_profile.argtypes = [ctypes.c_char_p]
    lib.axon_stop_nrt_profile.restype = ctypes.c_int64

    @contextlib.contextmanager
    def _hook(output_dir, device_ids):
        import jax

        jax.devices()
        if device_ids:
            ids = (ctypes.c_int64 * len(device_ids))(*device_ids)
            rc = lib.axon_start_nrt_profile(ids, len(device_ids))
        else:
            rc = lib.axon_start_nrt_profile(None, 0)
        if rc != 0:
            raise RuntimeError(f"axon_start_nrt_profile rc={rc}")
        try:
            yield
        finally:
            n = lib.axon_stop_nrt_profile(str(output_dir).encode())
            print(f"ntff profile: {n} file(s) written to {output_dir}")

    mod = types.ModuleType("antenv.axon_hooks")
    mod.get_axon_ntff_profile_hook = lambda: _hook
    mod.set_axon_ntff_profile_hook = lambda h: None
    sys.modules["antenv.axon_hooks"] = mod


def run(inputs, smooth, trace=False, **trace_kwargs):
    """Run on 8 cores; returns (y_full, BassKernelResults)."""
    if trace:
        _install_ntff_shim()
    nc = _get_nc()
    in_maps = _prep_in_maps(inputs, smooth)
    res = run_bass_kernel_spmd(
        nc, in_maps, list(range(NCORES)), trace=trace, **trace_kwargs
    )
    # yp [C, NU, R, B_LOC, QU] per core; batch axis is dim 3
    yp = np.concatenate([res.results[i]["y"] for i in range(NCORES)], axis=3)
    # y[b, t, c] with t = (qc*QU + ql)*R + m
    yf = (
        yp.astype(np.float32).transpose(3, 1, 4, 2, 0).reshape(B, T, C)
    )
    return np.ascontiguousarray(yf), res


def kernel(inputs, smooth):
    y, _ = run(inputs, smooth)
    return y


# revision 23
# speedup vs baseline: 1.0292x; 1.0292x over previous
"""Trainium2 Bass kernel: per-channel exponential moving average.

  a_t = k*x_t + (1-k)*a_{t-1},  a_{-1} = x_0   (per batch, per channel)

Full inputs: x [16, 8000, 512] f32, smooth [512] f32. Output [16, 8000, 512].

Strategy (8 NeuronCores, data-parallel over batch, 2 batches/core), with a
phase-decomposed scan that spreads the recurrence over all four engines:

  - Host pre-scales u = k*x, transposes to channel-major and deinterleaves
    time by R=8 phases, fp16, laid out [C, NU, B_LOC, R, QU] so each
    (channel-group, q-chunk) unit is one DMA call with 16KB/partition
    contiguous descriptors.
  - PE: block sums z[c,q] = sum_m d_c^(R-1-m) * U_m[c,q] via 8 accumulating
    matmuls with diagonal stationaries diag(d^pow) into PSUM (per-partition
    scale-and-add at 1 col/cycle fp16).
  - DVE: tensor_tensor_scan only over the R-decimated series
    A[q] = d^R * A[q-1] + z[q]  (T/R elements per channel instead of T).
  - Recon chain per phase i: out_i = d * out_{i-1} + U_i, out_{-1} = Ashift.
    The scale runs on ACT (activation Copy, per-partition scale) except
    phase 0 on DVE; the add runs on DVE (tensor_tensor, 2x fp16) except
    phase 3 on GpSimd. Units are processed in software-interleaved groups
    of 3 so the cross-engine chain never head-blocks an in-order stream.
  - Output is stored per phase-half (8KB descriptors) so out tiles drain
    early; y layout is [C, NU, R, B_LOC, QU].
  - Host re-interleaves the fp16 output phases and upcasts to f32.
"""
import numpy as np
from contextlib import ExitStack

import concourse.bass as bass
from concourse import bacc, masks, mybir
import concourse.tile as tile
from concourse.bass_utils import run_bass_kernel_spmd

B, T, C = 16, 8000, 512
NCORES = 8
B_LOC = B // NCORES  # batches per core
P = 128
CG = C // P          # channel groups (4)
R = 8                # phase decimation factor
RH = R // 2          # phases per output half
Q = T // R           # decimated length (1000)
NU = 2               # q-chunks per channel group
QU = Q // NU         # 500 (fits one psum bank as f32)
F32 = mybir.dt.float32
F16 = mybir.dt.float16

_CACHED_NC = None


def _build_nc():
    nc = bacc.Bacc(None, target_bir_lowering=False)
    x = nc.declare_dram_parameter("x", [C, NU, B_LOC, R, QU], F16, isOutput=False)
    dpow_d = nc.declare_dram_parameter("dpow_d", [P, CG, R], F32, isOutput=False)
    dR_d = nc.declare_dram_parameter("dR_d", [P, CG], F32, isOutput=False)
    dcol = nc.declare_dram_parameter("dcol", [P, CG], F32, isOutput=False)
    x0t = nc.declare_dram_parameter("x0t", [P, CG, B_LOC, 1], F32, isOutput=False)
    y = nc.declare_dram_parameter("y", [C, NU, R, B_LOC, QU], F16, isOutput=True)

    with tile.TileContext(nc) as tc, ExitStack() as ctx:
        singles = ctx.enter_context(tc.tile_pool(name="singles", bufs=1))
        inpool = ctx.enter_context(tc.tile_pool(name="inpool", bufs=7))
        outpool = ctx.enter_context(tc.tile_pool(name="outpool", bufs=6))
        apool = ctx.enter_context(tc.tile_pool(name="apool", bufs=2))
        tmppool = ctx.enter_context(tc.tile_pool(name="tmppool", bufs=4))
        zpool = ctx.enter_context(tc.tile_pool(name="zpool", bufs=4, space="PSUM"))

        # small params ride the HWDGE (sync) queue; bulk xin leads SWDGE
        dpow_sb = singles.tile([P, CG, R], F32)
        nc.sync.dma_start(out=dpow_sb[:], in_=dpow_d[:])
        dR_sb = singles.tile([P, CG], F32)
        nc.sync.dma_start(out=dR_sb[:], in_=dR_d[:])
        dcol_sb = singles.tile([P, CG], F32)
        nc.sync.dma_start(out=dcol_sb[:], in_=dcol[:])
        x0_sb = singles.tile([P, CG, B_LOC, 1], F32)
        nc.sync.dma_start(out=x0_sb[:], in_=x0t[:])

        ident = singles.tile([P, P], F32)
        masks.make_identity(nc, ident[:])
        diag = singles.tile([P, CG, R, P], F16)
        for cg in range(CG):
            for m in range(R):
                nc.vector.tensor_scalar(
                    diag[:, cg, m, :], ident[:],
                    dpow_sb[:, cg, m : m + 1], None,
                    mybir.AluOpType.mult,
                )
        ones = singles.tile([P, QU], F32)
        nc.vector.memset(ones[:], 1.0)
        dRbc = singles.tile([P, CG, QU], F32)
        for cg in range(CG):
            nc.scalar.activation(
                dRbc[:, cg, :], ones[:],
                mybir.ActivationFunctionType.Copy,
                scale=dR_sb[:, cg : cg + 1],
            )

        units = [(cg, qc) for qc in range(NU) for cg in range(CG)]
        prev_A = {}

        def stage_front(cg, qc):
            """DMA in, PE z-accumulation, A carry slot, scans."""
            cs = slice(cg * P, (cg + 1) * P)
            xin = inpool.tile([P, B_LOC, R, QU], F16, tag="xin", name="xin")
            nc.gpsimd.dma_start(out=xin[:], in_=x[cs, qc, :, :, :])
            z = zpool.tile([P, B_LOC, 512], F32, tag="z", name="z")
            for m in range(R):
                for b in range(B_LOC):
                    nc.tensor.matmul(
                        z[:, b, 0:QU],
                        diag[:, cg, m, :],
                        xin[:, b, m, :],
                        start=(m == 0),
                        stop=(m == R - 1),
                    )
            A = apool.tile([P, B_LOC, 1 + QU], F16, tag=f"A{cg}", name=f"A{cg}")
            carry = (
                x0_sb[:, cg, :, :] if qc == 0 else prev_A[cg][:, :, QU : QU + 1]
            )
            nc.gpsimd.tensor_copy(A[:, :, 0:1], carry)
            for b in range(B_LOC):
                init = (
                    x0_sb[:, cg, b, :]
                    if qc == 0
                    else prev_A[cg][:, b, QU : QU + 1]
                )
                nc.vector.tensor_tensor_scan(
                    A[:, b, 1 : 1 + QU],
                    dRbc[:, cg, :],
                    z[:, b, 0:QU],
                    init,
                    mybir.AluOpType.mult,
                    mybir.AluOpType.add,
                )
            prev_A[cg] = A
            return [cs, xin, A, None, None]

        DVE_TS = {0}      # phases whose scale runs on DVE (4x tensor_scalar)
        GP_TT = {3}       # phases whose add runs on gpsimd

        def recon_phase(st, cg, qc, i):
            cs, xin, A, halves, last = st
            if halves is None:
                halves = [
                    outpool.tile([P, RH, B_LOC, QU], F16, tag="outh", name="outh")
                    for _ in range(2)
                ]
                st[3] = halves
            prev = A[:, :, 0:QU] if i == 0 else last
            half = halves[i // RH]
            cur = half[:, i % RH, :, :]
            tmp = tmppool.tile([P, B_LOC, QU], F16, tag="tmp", name="tmp")
            if i in DVE_TS:
                nc.vector.tensor_scalar(
                    tmp[:], prev, dcol_sb[:, cg : cg + 1], None,
                    mybir.AluOpType.mult,
                )
            else:
                nc.scalar.activation(
                    tmp[:], prev,
                    mybir.ActivationFunctionType.Copy,
                    scale=dcol_sb[:, cg : cg + 1],
                )
            eng = nc.gpsimd if i in GP_TT else nc.vector
            eng.tensor_tensor(cur, tmp[:], xin[:, :, i, :], mybir.AluOpType.add)
            st[4] = cur
            if i % RH == RH - 1:
                h = i // RH
                nc.gpsimd.dma_start(
                    out=y[cs, qc, h * RH : (h + 1) * RH, :, :], in_=half[:]
                )

        # software-interleaved groups of 3 units, pipelined one group ahead:
        # group g+1's fronts (DMA/PE/scans) are emitted before group g's
        # recon so the PE and DMA streams never pause for reconstruction
        groups = [units[0:3], units[3:6], units[6:8]]
        pending = None
        for grp in groups:
            sts = [(stage_front(cg, qc), cg, qc) for cg, qc in grp]
            if pending is not None:
                for i in range(R):
                    for st, cg, qc in pending:
                        recon_phase(st, cg, qc, i)
            pending = sts
        for i in range(R):
            for st, cg, qc in pending:
                recon_phase(st, cg, qc, i)
    nc.compile()
    return nc


def _get_nc():
    global _CACHED_NC
    if _CACHED_NC is None:
        _CACHED_NC = _build_nc()
    return _CACHED_NC


def _prep_in_maps(inputs, smooth):
    f16 = np.dtype("float16")
    x = np.asarray(inputs, dtype=np.float32)
    sm = np.asarray(smooth, dtype=np.float32)
    k = np.clip(sm, 0.0, 1.0).astype(np.float32)
    d = (1.0 - k).astype(np.float32)
    # U[c, qc, b, m, ql] = (k*x)[b, (qc*QU+ql)*R + m, c]
    kxT = (x * k[None, None, :]).transpose(0, 2, 1)  # [B, C, T]
    U = np.ascontiguousarray(
        kxT.reshape(B, C, NU, QU, R).transpose(1, 2, 0, 4, 3)
    ).astype(f16)  # [C, NU, B, R, QU]
    dcol = np.ascontiguousarray(d.reshape(CG, P).T)  # [P, CG]
    d64 = d.astype(np.float64)
    # dpow[p, cg, m] = d_c^(R-1-m)
    pw = np.stack([d64 ** (R - 1 - m) for m in range(R)], axis=1)  # [C, R]
    dpow = np.ascontiguousarray(
        pw.astype(np.float32).reshape(CG, P, R).transpose(1, 0, 2)
    )
    dR = np.ascontiguousarray((d64 ** R).astype(np.float32).reshape(CG, P).T)
    x0 = x[:, 0, :].T.reshape(CG, P, B).transpose(1, 0, 2)[..., None]
    return [
        {
            "x": np.ascontiguousarray(U[:, :, i * B_LOC : (i + 1) * B_LOC]),
            "dpow_d": dpow,
            "dR_d": dR,
            "dcol": dcol,
            "x0t": np.ascontiguousarray(x0[:, :, i * B_LOC : (i + 1) * B_LOC, :]),
        }
        for i in range(NCORES)
    ]


def _install_ntff_shim():
    """Provide antenv.axon_hooks if the image lacks it (trace=True path).

    Replicates trn_agent_boot's ctypes NTFF hook against libaxon_pjrt.so.
    """
    import sys

    if "antenv.axon_hooks" in sys.modules:
        return
    try:
        import antenv.axon_hooks  # noqa: F401
        return
    except ImportError:
        pass
    import contextlib
    import ctypes
    import types

    so_path = "/opt/axon/libaxon_pjrt.so"
    try:
        lib = ctypes.CDLL(so_path)
    except OSError:
        return
    if not hasattr(lib, "axon_start_nrt_profile"):
        return
    lib.axon_start_nrt_profile.argtypes = [
        ctypes.POINTER(ctypes.c_int64),
        ctypes.c_size_t,
    ]
    lib.axon_start_nrt_profile.restype = ctypes.c_int64
    lib.axon_stop_nrt_profile.argtypes = [ctypes.c_char_p]
    lib.axon_stop_nrt_profile.restype = ctypes.c_int64

    @contextlib.contextmanager
    def _hook(output_dir, device_ids):
        import jax

        jax.devices()
        if device_ids:
            ids = (ctypes.c_int64 * len(device_ids))(*device_ids)
            rc = lib.axon_start_nrt_profile(ids, len(device_ids))
        else:
            rc = lib.axon_start_nrt_profile(None, 0)
        if rc != 0:
            raise RuntimeError(f"axon_start_nrt_profile rc={rc}")
        try:
            yield
        finally:
            n = lib.axon_stop_nrt_profile(str(output_dir).encode())
            print(f"ntff profile: {n} file(s) written to {output_dir}")

    mod = types.ModuleType("antenv.axon_hooks")
    mod.get_axon_ntff_profile_hook = lambda: _hook
    mod.set_axon_ntff_profile_hook = lambda h: None
    sys.modules["antenv.axon_hooks"] = mod


def run(inputs, smooth, trace=False, **trace_kwargs):
    """Run on 8 cores; returns (y_full, BassKernelResults)."""
    if trace:
        _install_ntff_shim()
    nc = _get_nc()
    in_maps = _prep_in_maps(inputs, smooth)
    res = run_bass_kernel_spmd(
        nc, in_maps, list(range(NCORES)), trace=trace, **trace_kwargs
    )
    # yp [C, NU, R, B_LOC, QU] per core; batch axis is dim 3
    yp = np.concatenate([res.results[i]["y"] for i in range(NCORES)], axis=3)
    # y[b, t, c] with t = (qc*QU + ql)*R + m
    yf = (
        yp.astype(np.float32).transpose(3, 1, 4, 2, 0).reshape(B, T, C)
    )
    return np.ascontiguousarray(yf), res


def kernel(inputs, smooth):
    y, _ = run(inputs, smooth)
    return y


# revision 27
# speedup vs baseline: 1.0884x; 1.0575x over previous
"""Trainium2 Bass kernel: per-channel exponential moving average.

  a_t = k*x_t + (1-k)*a_{t-1},  a_{-1} = x_0   (per batch, per channel)

Full inputs: x [16, 8000, 512] f32, smooth [512] f32. Output [16, 8000, 512].

Strategy (8 NeuronCores, data-parallel over batch, 2 batches/core), with a
phase-decomposed scan that spreads the recurrence over all four engines:

  - Host pre-scales u = k*x, transposes to channel-major and deinterleaves
    time by R=8 phases, fp16, laid out [C, NU, B_LOC, R, QU] so each
    (channel-group, q-chunk) unit is one DMA call with 16KB/partition
    contiguous descriptors.
  - PE: block sums z[c,q] = sum_m d_c^(R-1-m) * U_m[c,q] via 8 accumulating
    matmuls with diagonal stationaries diag(d^pow) into PSUM (per-partition
    scale-and-add at 1 col/cycle fp16).
  - DVE: tensor_tensor_scan only over the R-decimated series
    A[q] = d^R * A[q-1] + z[q]  (T/R elements per channel instead of T).
  - Recon chain per phase i: out_i = d * out_{i-1} + U_i, out_{-1} = Ashift.
    The scale runs on ACT (activation Copy, per-partition scale) except
    phase 0 on DVE; the add runs on DVE (tensor_tensor, 2x fp16) except
    phase 3 on GpSimd. Units are processed in software-interleaved groups
    of 3 so the cross-engine chain never head-blocks an in-order stream.
  - Output is stored per phase-half (8KB descriptors) so out tiles drain
    early; y layout is [C, NU, R, B_LOC, QU].
  - Host re-interleaves the fp16 output phases and upcasts to f32.
"""
import numpy as np
from contextlib import ExitStack

import concourse.bass as bass
from concourse import bacc, masks, mybir
import concourse.tile as tile
from concourse.bass_utils import run_bass_kernel_spmd

B, T, C = 16, 8000, 512
NCORES = 8
B_LOC = B // NCORES  # batches per core
P = 128
CG = C // P          # channel groups (4)
R = 8                # phase decimation factor
RH = R // 2          # phases per output half
Q = T // R           # decimated length (1000)
NU = 2               # q-chunks per channel group
QU = Q // NU         # 500 (fits one psum bank as f32)
F32 = mybir.dt.float32
F16 = mybir.dt.float16

_CACHED_NC = None


def _build_nc():
    nc = bacc.Bacc(None, target_bir_lowering=False)
    x = nc.declare_dram_parameter("x", [C, NU, B_LOC, R, QU], F16, isOutput=False)
    dpow_d = nc.declare_dram_parameter("dpow_d", [P, CG, R], F32, isOutput=False)
    dR_d = nc.declare_dram_parameter("dR_d", [P, CG], F32, isOutput=False)
    dcol = nc.declare_dram_parameter("dcol", [P, CG], F32, isOutput=False)
    x0t = nc.declare_dram_parameter("x0t", [P, CG, B_LOC, 1], F32, isOutput=False)
    y = nc.declare_dram_parameter("y", [C, NU, R, B_LOC, QU], F16, isOutput=True)

    with tile.TileContext(nc) as tc, ExitStack() as ctx:
        singles = ctx.enter_context(tc.tile_pool(name="singles", bufs=1))
        inpool = ctx.enter_context(tc.tile_pool(name="inpool", bufs=7))
        outpool = ctx.enter_context(tc.tile_pool(name="outpool", bufs=6))
        apool = ctx.enter_context(tc.tile_pool(name="apool", bufs=2))
        tmppool = ctx.enter_context(tc.tile_pool(name="tmppool", bufs=4))
        zpool = ctx.enter_context(tc.tile_pool(name="zpool", bufs=4, space="PSUM"))

        # small params ride the HWDGE (sync) queue; bulk xin leads SWDGE
        dpow_sb = singles.tile([P, CG, R], F32)
        nc.sync.dma_start(out=dpow_sb[:], in_=dpow_d[:])
        dR_sb = singles.tile([P, CG], F32)
        nc.sync.dma_start(out=dR_sb[:], in_=dR_d[:])
        dcol_sb = singles.tile([P, CG], F32)
        nc.sync.dma_start(out=dcol_sb[:], in_=dcol[:])
        x0_sb = singles.tile([P, CG, B_LOC, 1], F32)
        nc.sync.dma_start(out=x0_sb[:], in_=x0t[:])

        ident = singles.tile([P, P], F32)
        masks.make_identity(nc, ident[:])
        diag = singles.tile([P, CG, R, P], F16)
        for cg in range(CG):
            for m in range(R):
                nc.vector.tensor_scalar(
                    diag[:, cg, m, :], ident[:],
                    dpow_sb[:, cg, m : m + 1], None,
                    mybir.AluOpType.mult,
                )
        ones = singles.tile([P, QU], F32)
        nc.vector.memset(ones[:], 1.0)
        dRbc = singles.tile([P, CG, QU], F32)
        for cg in range(CG):
            nc.scalar.activation(
                dRbc[:, cg, :], ones[:],
                mybir.ActivationFunctionType.Copy,
                scale=dR_sb[:, cg : cg + 1],
            )

        units = [(cg, qc) for qc in range(NU) for cg in range(CG)]
        prev_A = {}

        def stage_front(cg, qc):
            """DMA in, PE z-accumulation, A carry slot, scans."""
            cs = slice(cg * P, (cg + 1) * P)
            xin = inpool.tile([P, B_LOC, R, QU], F16, tag="xin", name="xin")
            nc.gpsimd.dma_start(out=xin[:], in_=x[cs, qc, :, :, :])
            z = zpool.tile([P, B_LOC, 512], F32, tag="z", name="z")
            for m in range(R):
                for b in range(B_LOC):
                    nc.tensor.matmul(
                        z[:, b, 0:QU],
                        diag[:, cg, m, :],
                        xin[:, b, m, :],
                        start=(m == 0),
                        stop=(m == R - 1),
                    )
            A = apool.tile([P, B_LOC, 1 + QU], F16, tag=f"A{cg}", name=f"A{cg}")
            carry = (
                x0_sb[:, cg, :, :] if qc == 0 else prev_A[cg][:, :, QU : QU + 1]
            )
            nc.gpsimd.tensor_copy(A[:, :, 0:1], carry)
            for b in range(B_LOC):
                init = (
                    x0_sb[:, cg, b, :]
                    if qc == 0
                    else prev_A[cg][:, b, QU : QU + 1]
                )
                nc.vector.tensor_tensor_scan(
                    A[:, b, 1 : 1 + QU],
                    dRbc[:, cg, :],
                    z[:, b, 0:QU],
                    init,
                    mybir.AluOpType.mult,
                    mybir.AluOpType.add,
                )
            prev_A[cg] = A
            return [cs, xin, A, None, None]

        def recon_phase(st, cg, qc, i, dve_ts=frozenset({0}), gp_tt=frozenset({3})):
            cs, xin, A, halves, last = st
            if halves is None:
                halves = [
                    outpool.tile([P, RH, B_LOC, QU], F16, tag="outh", name="outh")
                    for _ in range(2)
                ]
                st[3] = halves
            prev = A[:, :, 0:QU] if i == 0 else last
            half = halves[i // RH]
            cur = half[:, i % RH, :, :]
            tmp = tmppool.tile([P, B_LOC, QU], F16, tag="tmp", name="tmp")
            if i in dve_ts:
                nc.vector.tensor_scalar(
                    tmp[:], prev, dcol_sb[:, cg : cg + 1], None,
                    mybir.AluOpType.mult,
                )
            else:
                nc.scalar.activation(
                    tmp[:], prev,
                    mybir.ActivationFunctionType.Copy,
                    scale=dcol_sb[:, cg : cg + 1],
                )
            eng = nc.gpsimd if i in gp_tt else nc.vector
            eng.tensor_tensor(cur, tmp[:], xin[:, :, i, :], mybir.AluOpType.add)
            st[4] = cur
            if i % RH == RH - 1:
                h = i // RH
                nc.gpsimd.dma_start(
                    out=y[cs, qc, h * RH : (h + 1) * RH, :, :], in_=half[:]
                )

        # software-interleaved groups of 3 units, pipelined one group ahead:
        # group g+1's fronts (DMA/PE/scans) are emitted before group g's
        # recon so the PE and DMA streams never pause for reconstruction
        groups = [units[0:3], units[3:6], units[6:8]]
        pending = None
        for grp in groups:
            sts = [(stage_front(cg, qc), cg, qc) for cg, qc in grp]
            if pending is not None:
                for i in range(R):
                    for st, cg, qc in pending:
                        recon_phase(st, cg, qc, i)
            pending = sts
        # tail group: no front work left to overlap, so spread the recon
        # across disjoint engines — unit 6 self-contained on DVE, unit 7
        # ACT-led with two adds on the otherwise-idle gpsimd
        ALL = frozenset(range(R))
        tail_maps = [(ALL, frozenset()), (frozenset({0}), frozenset({3, 5}))]
        for i in range(R):
            for (st, cg, qc), (dts, gtt) in zip(pending, tail_maps):
                recon_phase(st, cg, qc, i, dve_ts=dts, gp_tt=gtt)
    nc.compile()
    return nc


def _get_nc():
    global _CACHED_NC
    if _CACHED_NC is None:
        _CACHED_NC = _build_nc()
    return _CACHED_NC


def _prep_in_maps(inputs, smooth):
    f16 = np.dtype("float16")
    x = np.asarray(inputs, dtype=np.float32)
    sm = np.asarray(smooth, dtype=np.float32)
    k = np.clip(sm, 0.0, 1.0).astype(np.float32)
    d = (1.0 - k).astype(np.float32)
    # U[c, qc, b, m, ql] = (k*x)[b, (qc*QU+ql)*R + m, c]
    kxT = (x * k[None, None, :]).transpose(0, 2, 1)  # [B, C, T]
    U = np.ascontiguousarray(
        kxT.reshape(B, C, NU, QU, R).transpose(1, 2, 0, 4, 3)
    ).astype(f16)  # [C, NU, B, R, QU]
    dcol = np.ascontiguousarray(d.reshape(CG, P).T)  # [P, CG]
    d64 = d.astype(np.float64)
    # dpow[p, cg, m] = d_c^(R-1-m)
    pw = np.stack([d64 ** (R - 1 - m) for m in range(R)], axis=1)  # [C, R]
    dpow = np.ascontiguousarray(
        pw.astype(np.float32).reshape(CG, P, R).transpose(1, 0, 2)
    )
    dR = np.ascontiguousarray((d64 ** R).astype(np.float32).reshape(CG, P).T)
    x0 = x[:, 0, :].T.reshape(CG, P, B).transpose(1, 0, 2)[..., None]
    return [
        {
            "x": np.ascontiguousarray(U[:, :, i * B_LOC : (i + 1) * B_LOC]),
            "dpow_d": dpow,
            "dR_d": dR,
            "dcol": dcol,
            "x0t": np.ascontiguousarray(x0[:, :, i * B_LOC : (i + 1) * B_LOC, :]),
        }
        for i in range(NCORES)
    ]


def _install_ntff_shim():
    """Provide antenv.axon_hooks if the image lacks it (trace=True path).

    Replicates trn_agent_boot's ctypes NTFF hook against libaxon_pjrt.so.
    """
    import sys

    if "antenv.axon_hooks" in sys.modules:
        return
    try:
        import antenv.axon_hooks  # noqa: F401
        return
    except ImportError:
        pass
    import contextlib
    import ctypes
    import types

    so_path = "/opt/axon/libaxon_pjrt.so"
    try:
        lib = ctypes.CDLL(so_path)
    except OSError:
        return
    if not hasattr(lib, "axon_start_nrt_profile"):
        return
    lib.axon_start_nrt_profile.argtypes = [
        ctypes.POINTER(ctypes.c_int64),
        ctypes.c_size_t,
    ]
    lib.axon_start_nrt_profile.restype = ctypes.c_int64
    lib.axon_stop_nrt_profile.argtypes = [ctypes.c_char_p]
    lib.axon_stop_nrt_profile.restype = ctypes.c_int64

    @contextlib.contextmanager
    def _hook(output_dir, device_ids):
        import jax

        jax.devices()
        if device_ids:
            ids = (ctypes.c_int64 * len(device_ids))(*device_ids)
            rc = lib.axon_start_nrt_profile(ids, len(device_ids))
        else:
            rc = lib.axon_start_nrt_profile(None, 0)
        if rc != 0:
            raise RuntimeError(f"axon_start_nrt_profile rc={rc}")
        try:
            yield
        finally:
            n = lib.axon_stop_nrt_profile(str(output_dir).encode())
            print(f"ntff profile: {n} file(s) written to {output_dir}")

    mod = types.ModuleType("antenv.axon_hooks")
    mod.get_axon_ntff_profile_hook = lambda: _hook
    mod.set_axon_ntff_profile_hook = lambda h: None
    sys.modules["antenv.axon_hooks"] = mod


def run(inputs, smooth, trace=False, **trace_kwargs):
    """Run on 8 cores; returns (y_full, BassKernelResults)."""
    if trace:
        _install_ntff_shim()
    nc = _get_nc()
    in_maps = _prep_in_maps(inputs, smooth)
    res = run_bass_kernel_spmd(
        nc, in_maps, list(range(NCORES)), trace=trace, **trace_kwargs
    )
    # yp [C, NU, R, B_LOC, QU] per core; batch axis is dim 3
    yp = np.concatenate([res.results[i]["y"] for i in range(NCORES)], axis=3)
    # y[b, t, c] with t = (qc*QU + ql)*R + m
    yf = (
        yp.astype(np.float32).transpose(3, 1, 4, 2, 0).reshape(B, T, C)
    )
    return np.ascontiguousarray(yf), res


def kernel(inputs, smooth):
    y, _ = run(inputs, smooth)
    return y
